# revision 20
# baseline (speedup 1.0000x reference)
"""GCN layer (gnn_message_passing) on 8 Trainium2 NeuronCores.

Math (matches torch_geometric GCNConv defaults / the jax reference):
    deg[d]  = sum_{e: dst=d} w_e + 1                      (self loop w=1)
    dinv    = deg^-1/2
    h       = x @ W
    out[d]  = relu( dinv[d] * ( sum_{e->d} w_e * dinv[src_e] * h[src_e]
                                + dinv[d] * h[d] )  + b )

Distribution: nodes sharded contiguously across 8 cores (6250/core), then
PERMUTED host-side into 50 tiles x 128 slots per core by a balanced
packing so that nearly every (tile, src-half) bin holds <= 1024 edges
(8 blocks of 128).  Edges are partitioned by dst owner.

Per core program (SPMD, one compiled NEFF):
  1. deg -> dinv via padded row table + DVE reduce.
  2. h = x@W per tile (PE f32); hs = dinv*h (bf16) written to two DRAM
     shards (pool A = tiles 0-24, pool B = 25-49); hsb = dinv^2*h + b in
     SBUF.  AllGather of pool A is issued right after tile 24 so it
     overlaps the second half of phase 2; pool B follows.
  3. AllGather'd tables copied to plain DRAM (SWDGE reads from Shared
     space are slow).
  4. Main pass over this core's edges in blocks of 128 (uniform
     (tile, half) bin structure across cores):
       - dma_gather 16 blocks/call (SWDGE), rotating queue 0-3 so all
         four Q7 descriptor-generator core pairs run concurrently
       - one-hot S[e, dst_local] = w_e in ONE DVE tensor_scalar
       - PE matmul agg[dst,f] += S^T @ hs_gathered accumulated in PSUM
       - epilogue per tile: relu(dinv*agg + hsb) -> out rows.
"""

import math
import os
import sys

import numpy as np

P = 128           # partition / tile size
NCORES = 8
G_TILES = 4       # dst tiles per PSUM group
MAXBLK = 8        # max 128-edge blocks per dma_gather call
NT = 50           # tiles per core
NP_ = NT * P      # padded nodes per core (6400)
POOL = NP_ // 2   # nodes per gather-table pool (3200)
POOL_T = NT // 2  # tiles per pool

_CACHE = {}


def _import_concourse():
    try:
        import concourse.bass  # noqa: F401
        return
    except ImportError:
        pass
    for p in ("/opt/trn_rl_repo", "/root/.axon_site/_ro/trn_rl_repo"):
        if os.path.isdir(p) and p not in sys.path:
            sys.path.insert(0, p)
    import concourse.bass  # noqa: F401


def _ceil(a, b):
    return -(-a // b)


def _pack_pool(d0, d1):
    """Pack len(d0) nodes (with per-half in-degree loads) into POOL_T
    tiles of 128 slots, balancing both halves' loads (greedy LPT on
    max(L0+d0, L1+d1) with a slot-count cap).  Returns tile per node."""
    n = len(d0)
    assert n == POOL_T * P
    order = np.argsort(-(d0 + d1), kind="stable")
    L0 = np.zeros(POOL_T)
    L1 = np.zeros(POOL_T)
    cnt = np.zeros(POOL_T, np.int64)
    tile_of = np.empty(n, np.int64)
    for i in order:
        c0 = L0 + d0[i]
        c1 = L1 + d1[i]
        score = np.maximum(c0, c1)
        score[cnt >= P] = 1e18
        t = int(np.argmin(score))
        tile_of[i] = t
        L0[t] = c0[t]
        L1[t] = c1[t]
        cnt[t] += 1
    return tile_of


def _preprocess(x, edge_index, edge_weight, W, b):
    """Shard + reorganize inputs on host. Returns (cfg, in_maps, g_idx)."""
    x = np.asarray(x, dtype=np.float32)
    W = np.asarray(W, dtype=np.float32)
    b = np.asarray(b, dtype=np.float32)
    ei = np.asarray(edge_index)
    ew = np.asarray(edge_weight, dtype=np.float32)

    N, C = x.shape
    F = W.shape[1]
    assert C % P == 0 and F == P
    CH = C // P
    PER = _ceil(N, NCORES)
    HALF = NCORES * POOL              # gather-table rows per pool table
    assert HALF <= 32768, "int16 gather index range exceeded"
    assert NP_ >= PER

    src = ei[0].astype(np.int64)
    dst = ei[1].astype(np.int64)

    o_src = src // PER
    lsrc = src - o_src * PER                    # 0..PER-1 within owner
    owner = dst // PER
    HALF_REAL = PER // 2
    half_e = (lsrc >= HALF_REAL).astype(np.int64)

    # per-node in-degree by src half -> packing loads
    d0 = np.zeros(N, np.int64)
    d1 = np.zeros(N, np.int64)
    np.add.at(d0, dst[half_e == 0], 1)
    np.add.at(d1, dst[half_e == 1], 1)

    # pack each (core, pool) into POOL_T tiles x 128 slots
    slot_of = np.zeros(N, np.int64)
    for c in range(NCORES):
        base = c * PER
        for pool in (0, 1):
            if pool == 0:
                nodes = np.arange(base, base + HALF_REAL)
            else:
                nodes = np.arange(base + HALF_REAL, base + PER)
            nfake = POOL - len(nodes)
            dd0 = np.concatenate([d0[nodes], np.zeros(nfake, np.int64)])
            dd1 = np.concatenate([d1[nodes], np.zeros(nfake, np.int64)])
            tile_of = _pack_pool(dd0, dd1)
            # slot within pool: tile*128 + fill position
            slot_local = np.empty(len(tile_of), np.int64)
            for t in range(POOL_T):
                m = np.nonzero(tile_of == t)[0]
                slot_local[m] = t * P + np.arange(len(m))
            slot_of[nodes] = pool * POOL + slot_local[:len(nodes)]

    slot_src = slot_of[src]
    gsrc = o_src * POOL + (slot_src % POOL)     # row within pool table
    half = slot_src // POOL                     # which table (0/1)
    slot_dst = slot_of[dst]
    tile_g = slot_dst // P
    dloc = slot_dst % P

    # per-core counts per (tile, half) -> unified block structure
    cnt = np.zeros((NCORES, NT, 2), np.int64)
    np.add.at(cnt, (owner, tile_g, half), 1)
    nb = _ceil(cnt, P).max(axis=0)              # [NT, 2]
    for t in range(NT):
        for h in (0, 1):
            if nb[t][h] == 0:
                nb[t][h] = 1

    # block stream: sweep all tiles' h0 runs (table A), then all h1 runs
    # (table B).  The h0 sweep streams while AllGather-B is still in
    # flight; each tile's h0 partial is parked in SBUF between sweeps.
    blocks = []                                  # (tile, half)
    base_blk = np.zeros((NT, 2), np.int64)
    for h in (0, 1):
        for t in range(NT):
            base_blk[t, h] = len(blocks)
            blocks.extend([(t, h)] * int(nb[t, h]))
    NBLK = len(blocks)
    NIDX = NBLK * P

    # calls: chunk maximal same-half runs into <=MAXBLK pieces
    calls = []                                   # (half, b0, nblk)
    i = 0
    while i < NBLK:
        h = blocks[i][1]
        j = i
        while j < NBLK and blocks[j][1] == h:
            j += 1
        k = i
        while k < j:
            n = min(MAXBLK, j - k)
            calls.append((h, k, n))
            k += n
        i = j
    seg_first = {}
    seg_last = {}
    for bi, (t, h) in enumerate(blocks):
        seg_first.setdefault((t, h), bi)
        seg_last[(t, h)] = bi

    # degree-table width (uniform across cores): max in-degree + 1 self
    deg_cnt = np.bincount(dst, minlength=N)
    PW = int(deg_cnt.max()) + 1

    import ml_dtypes
    in_maps = []
    B128 = np.tile(b[None, :], (P, 1)).astype(np.float32)
    IDENT = np.eye(P, dtype=np.float32).astype(ml_dtypes.bfloat16)

    base_flat = base_blk.reshape(-1)
    for c in range(NCORES):
        m = owner == c
        s_c = gsrc[m]
        h_c = half[m]
        t_c = tile_g[m]
        dl_c = dloc[m]
        w_c = ew[m]
        ld_c = slot_dst[m]

        # ---- edge stream positions (ascending gsrc within each bin for
        # HBM row locality during the gather) --------------------------
        key = t_c * 2 + h_c
        order = np.argsort(key * (1 << 15) + s_c, kind="stable")
        sk = key[order]
        grp_off = np.arange(len(sk)) - np.searchsorted(sk, sk)
        pos = base_flat[sk] * P + grp_off        # position in edge stream

        relidx = np.zeros(NIDX, np.int16)
        relidx[pos] = s_c[order].astype(np.int16)
        idx16 = np.ascontiguousarray(
            np.tile(relidx.reshape(NIDX // 16, 16).T, (P // 16, 1)))
        # host-built one-hot S blocks: S[e, dst_local] = w_e, laid out
        # [P (edge slot), NBLK*P] with block bi at cols [bi*P,(bi+1)*P)
        sblk = np.zeros((P, NBLK * P), np.float32)
        sblk[pos % P, (pos // P) * P + dl_c[order]] = w_c[order]
        sblk = sblk.astype(ml_dtypes.bfloat16)

        # ---- degree table [P, NT*PW]: row l%128, cols (l//128)*PW+j ---
        wpad = np.zeros((P, NT * PW), np.float32)
        o2 = np.argsort(ld_c, kind="stable")
        lds = ld_c[o2]
        ws = w_c[o2]
        off2 = np.arange(len(lds)) - np.searchsorted(lds, lds)
        wpad[lds % P, (lds // P) * PW + off2] = ws
        alln = np.arange(NP_)
        wpad[alln % P, (alln // P) * PW + np.minimum(
            np.bincount(ld_c, minlength=NP_), PW - 1)] = 1.0  # self loop

        # ---- xT shard [C, NP_] (node rows permuted to slots) ----------
        lo = c * PER
        hi = min((c + 1) * PER, N)
        xc = np.zeros((NP_, C), np.float32)
        xc[slot_of[lo:hi]] = x[lo:hi]
        xT = np.ascontiguousarray(xc.T).astype(ml_dtypes.bfloat16)

        in_maps.append({
            "xT": xT,
            "w_in": W.astype(ml_dtypes.bfloat16),
            "bias128": B128,
            "ident": IDENT,
            "wpad": wpad,
            "sblk": sblk,
            "idx16": idx16,
        })

    cfg = dict(N=N, C=C, F=F, CH=CH, PER=PER, HALF=HALF,
               NBLK=NBLK, NIDX=NIDX, PW=PW,
               nb=tuple(map(tuple, nb.tolist())),
               blocks=tuple(blocks), calls=tuple(calls),
               seg_first=tuple(sorted(seg_first.items())),
               seg_last=tuple(sorted(seg_last.items())))
    # full-output gather index: node -> row in concat-of-core outputs
    g_idx = (np.arange(N) // PER) * NP_ + slot_of
    return cfg, in_maps, g_idx


def _build(cfg):
    _import_concourse()
    from concourse import bacc, mybir, tile
    dt = mybir.dt
    Alu = mybir.AluOpType
    Act = mybir.ActivationFunctionType
    X = mybir.AxisListType.X

    C, F, CH = cfg["C"], cfg["F"], cfg["CH"]
    HALF, NBLK, NIDX, PW = cfg["HALF"], cfg["NBLK"], cfg["NIDX"], cfg["PW"]
    blocks = cfg["blocks"]
    calls = cfg["calls"]
    seg_first = dict(cfg["seg_first"])
    seg_last = dict(cfg["seg_last"])

    nc = bacc.Bacc("TRN2", target_bir_lowering=False, debug=False,
                   num_devices=NCORES, num_swdge_queues=4)

    xT_d = nc.dram_tensor("xT", [C, NP_], dt.bfloat16, kind="ExternalInput")
    W_d = nc.dram_tensor("w_in", [C, F], dt.bfloat16, kind="ExternalInput")
    B_d = nc.dram_tensor("bias128", [P, F], dt.float32, kind="ExternalInput")
    I_d = nc.dram_tensor("ident", [P, P], dt.bfloat16, kind="ExternalInput")
    wpad_d = nc.dram_tensor("wpad", [P, NT * PW], dt.float32,
                            kind="ExternalInput")
    sblk_d = nc.dram_tensor("sblk", [P, NBLK * P], dt.bfloat16,
                            kind="ExternalInput")
    idx_d = nc.dram_tensor("idx16", [P, NIDX // 16], dt.int16,
                           kind="ExternalInput")
    out_d = nc.dram_tensor("out", [NP_, F], dt.float32, kind="ExternalOutput")
    hs_sh = [
        nc.dram_tensor("hs_shA", [POOL, F], dt.bfloat16),
        nc.dram_tensor("hs_shB", [POOL, F], dt.bfloat16),
    ]
    hs_ag = [
        nc.dram_tensor("hs_agA", [HALF, F], dt.bfloat16, addr_space="Shared"),
        nc.dram_tensor("hs_agB", [HALF, F], dt.bfloat16, addr_space="Shared"),
    ]
    # gather from plain-DRAM copies — SWDGE reads from Shared space are slow
    hs_tab = [
        nc.dram_tensor("hs_fullA", [HALF, F], dt.bfloat16),
        nc.dram_tensor("hs_fullB", [HALF, F], dt.bfloat16),
    ]

    with tile.TileContext(nc) as tc:
        with (
            tc.tile_pool(name="const", bufs=1) as cpool,
            tc.tile_pool(name="psum", bufs=8, space="PSUM") as ppool,
            tc.tile_pool(name="work", bufs=8) as wpool,
            tc.tile_pool(name="gather", bufs=32) as gpool,
            tc.tile_pool(name="sbuild", bufs=12) as spool,
            tc.tile_pool(name="xt", bufs=4) as xpool,
        ):
            # ---------------- const loads ------------------------------
            W_sb = []
            for ch in range(CH):
                t2 = cpool.tile([P, F], dt.bfloat16, tag=f"W{ch}")
                nc.sync.dma_start(t2[:], W_d[ch * P:(ch + 1) * P, :])
                W_sb.append(t2)
            B_sb = cpool.tile([P, F], dt.float32, tag="B")
            nc.sync.dma_start(B_sb[:], B_d[:])
            I_sb = cpool.tile([P, P], dt.bfloat16, tag="I")
            nc.sync.dma_start(I_sb[:], I_d[:])
            wpad_sb = cpool.tile([P, NT * PW], dt.float32, tag="wpad")
            nc.sync.dma_start(wpad_sb[:], wpad_d[:])
            idx_sb = cpool.tile([P, NIDX // 16], dt.int16, tag="idx")
            nc.sync.dma_start(idx_sb[:], idx_d[:])
            part_sb = cpool.tile([P, NT * F], dt.float32, tag="part")
            hs_tiles = [cpool.tile([P, F], dt.bfloat16, tag=f"hs{t}",
                                   name=f"hs{t}")
                        for t in range(NT)]
            dinv_sb = cpool.tile([P, NT], dt.float32, tag="dinv")

            # ---------------- phase 1: degrees -------------------------
            degt = wpool.tile([P, NT], dt.float32, tag="deg")
            for t in range(NT):
                nc.vector.reduce_sum(degt[:, t:t + 1],
                                     wpad_sb[:, t * PW:(t + 1) * PW], X)
            rec = wpool.tile([P, NT], dt.float32, tag="rec")
            nc.vector.reciprocal(rec[:], degt[:])
            nc.scalar.activation(dinv_sb[:], rec[:], Act.Sqrt)

            # ---------------- phase 2: h = xW, hs, hsb -----------------
            # xT loaded in octets of 8 tiles (fewer, bigger DMAs);
            # per-pool: finish pool's tiles then AllGather that pool.
            def phase2_tiles(t0, t1):
                t = t0
                while t < t1:
                    span = min(8, t1 - t)
                    xts = []
                    for ch in range(CH):
                        xt_t = xpool.tile([P, 8 * P], dt.bfloat16, tag="xt")
                        nc.sync.dma_start(
                            xt_t[:, :span * P],
                            xT_d[ch * P:(ch + 1) * P,
                                 t * P:(t + span) * P])
                        xts.append(xt_t)
                    for j in range(span):
                        tt = t + j
                        ph = ppool.tile([P, F], dt.float32, tag="psum")
                        for ch in range(CH):
                            nc.tensor.matmul(
                                ph[:], xts[ch][:, j * P:(j + 1) * P],
                                W_sb[ch][:], start=(ch == 0),
                                stop=(ch == CH - 1))
                        hsbf = hs_tiles[tt]
                        nc.vector.tensor_scalar(
                            hsbf[:], ph[:], dinv_sb[:, tt:tt + 1],
                            None, Alu.mult)
                        pool, tl = divmod(tt, POOL_T)
                        eng = nc.scalar if tt % 2 == 0 else nc.sync
                        eng.dma_start(
                            hs_sh[pool][tl * P:(tl + 1) * P, :], hsbf[:])
                    t += span

            for hh in (0, 1):
                phase2_tiles(hh * POOL_T, (hh + 1) * POOL_T)
                # -------- phase 3: AllGather this pool (<1MB/rank) -----
                nc.gpsimd.collective_compute(
                    "AllGather", Alu.bypass,
                    replica_groups=[list(range(NCORES))],
                    ins=[hs_sh[hh].ap().opt()],
                    outs=[hs_ag[hh].ap().opt()],
                )
                # copy off the gpsimd queue so it never head-of-line
                # blocks the gather stream; split halves across two queues
                HH = HALF // 2
                nc.sync.dma_start(hs_tab[hh][:HH, :], hs_ag[hh][:HH, :])
                nc.scalar.dma_start(hs_tab[hh][HH:, :], hs_ag[hh][HH:, :])

            # ---------------- phase 4: gather + segment matmul ---------
            agg = {}
            for ci, (h, b0, nbc) in enumerate(calls):
                gb = gpool.tile([P, MAXBLK, F], dt.bfloat16, tag="gb")
                nc.gpsimd.dma_gather(
                    gb[:, :nbc, :],
                    hs_tab[h].ap(),
                    idx_sb[:, b0 * (P // 16):(b0 + nbc) * (P // 16)],
                    nbc * P, nbc * P, F, single_packet=False,
                    queue_num=ci % 4)
                sb = spool.tile([P, MAXBLK, P], dt.bfloat16, tag="S")
                nc.scalar.dma_start(
                    sb[:, :nbc, :],
                    sblk_d[:, b0 * P:(b0 + nbc) * P])
                for j in range(nbc):
                    bi = b0 + j
                    t, hh = blocks[bi]
                    S = sb[:, j, :]
                    first = bi == seg_first[(t, hh)]
                    last = bi == seg_last[(t, hh)]
                    if first:
                        agg[(t, hh)] = ppool.tile([P, F], dt.float32,
                                                  tag="psum",
                                                  name=f"agg{t}_{hh}")
                    a = agg[(t, hh)]
                    inj = hh == 0 and first
                    nc.tensor.matmul(a[:], S, gb[:, j, :], start=first,
                                     stop=(last and not inj))
                    if inj:
                        # self-loop: agg += I^T @ hs_t  (adds dinv*h_t)
                        nc.tensor.matmul(a[:], I_sb[:], hs_tiles[t][:],
                                         start=False, stop=last)
                    if not last:
                        continue
                    if hh == 0:
                        # park sweep-A partial: dinv*(aggA+hs) + b
                        nc.vector.scalar_tensor_tensor(
                            part_sb[:, t * F:(t + 1) * F], a[:],
                            dinv_sb[:, t:t + 1], B_sb[:],
                            Alu.mult, Alu.add)
                    else:
                        res = wpool.tile([P, F], dt.float32, tag="res")
                        nc.vector.scalar_tensor_tensor(
                            res[:], a[:], dinv_sb[:, t:t + 1],
                            part_sb[:, t * F:(t + 1) * F],
                            Alu.mult, Alu.add)
                        ot = wpool.tile([P, F], dt.float32, tag="ot")
                        nc.scalar.activation(ot[:], res[:], Act.Relu)
                        nc.sync.dma_start(out_d[t * P:(t + 1) * P, :],
                                          ot[:])

    nc.compile()
    return nc


# knobs test.py can flip
TRACE = False
LAST_EXEC_NS = None
LAST_TRACE_PATH = None


def _cfg_key(cfg):
    return (cfg["N"], cfg["C"], cfg["F"], cfg["NBLK"], cfg["PW"],
            cfg["nb"], cfg["calls"])


def kernel(x, edge_index, edge_weight, W, b):
    global LAST_EXEC_NS, LAST_TRACE_PATH
    _import_concourse()
    from concourse import bass_utils

    cfg, in_maps, g_idx = _preprocess(x, edge_index, edge_weight, W, b)
    key = _cfg_key(cfg)
    nc = _CACHE.get(key)
    if nc is None:
        nc = _build(cfg)
        _CACHE[key] = nc

    res = bass_utils.run_bass_kernel_spmd(
        nc, in_maps, core_ids=list(range(NCORES)), trace=TRACE)
    LAST_EXEC_NS = res.exec_time_ns
    if res.instructions_and_trace is not None:
        LAST_TRACE_PATH = res.instructions_and_trace[1]

    flat = np.concatenate([res.results[c]["out"] for c in range(NCORES)],
                          axis=0)
    return np.ascontiguousarray(flat[g_idx])


# revision 21
# speedup vs baseline: 1.1108x; 1.1108x over previous
"""GCN layer (gnn_message_passing) on 8 Trainium2 NeuronCores.

Math (matches torch_geometric GCNConv defaults / the jax reference):
    deg[d]  = sum_{e: dst=d} w_e + 1                      (self loop w=1)
    dinv    = deg^-1/2
    h       = x @ W
    out[d]  = relu( dinv[d] * ( sum_{e->d} w_e * dinv[src_e] * h[src_e]
                                + dinv[d] * h[d] )  + b )

Distribution: nodes sharded contiguously across 8 cores (6250/core), then
PERMUTED host-side into 50 tiles x 128 slots per core by a balanced
packing so that nearly every (tile, src-half) bin holds <= 1024 edges
(8 blocks of 128).  Edges are partitioned by dst owner.

Per core program (SPMD, one compiled NEFF):
  1. deg -> dinv via padded row table + DVE reduce.
  2. h = x@W per tile (PE f32); hs = dinv*h (bf16) written to two DRAM
     shards (pool A = tiles 0-24, pool B = 25-49); hsb = dinv^2*h + b in
     SBUF.  AllGather of pool A is issued right after tile 24 so it
     overlaps the second half of phase 2; pool B follows.
  3. AllGather'd tables copied to plain DRAM (SWDGE reads from Shared
     space are slow).
  4. Main pass over this core's edges in blocks of 128 (uniform
     (tile, half) bin structure across cores):
       - dma_gather 16 blocks/call (SWDGE), rotating queue 0-3 so all
         four Q7 descriptor-generator core pairs run concurrently
       - one-hot S[e, dst_local] = w_e in ONE DVE tensor_scalar
       - PE matmul agg[dst,f] += S^T @ hs_gathered accumulated in PSUM
       - epilogue per tile: relu(dinv*agg + hsb) -> out rows.
"""

import math
import os
import sys

import numpy as np

P = 128           # partition / tile size
NCORES = 8
G_TILES = 4       # dst tiles per PSUM group
MAXBLK = 8        # max 128-edge blocks per dma_gather call
NT = 50           # tiles per core
NP_ = NT * P      # padded nodes per core (6400)
POOL = NP_ // 2   # nodes per gather-table pool (3200)
POOL_T = NT // 2  # tiles per pool

_CACHE = {}


def _import_concourse():
    try:
        import concourse.bass  # noqa: F401
        return
    except ImportError:
        pass
    for p in ("/opt/trn_rl_repo", "/root/.axon_site/_ro/trn_rl_repo"):
        if os.path.isdir(p) and p not in sys.path:
            sys.path.insert(0, p)
    import concourse.bass  # noqa: F401


def _ceil(a, b):
    return -(-a // b)


def _pack_pool(d0, d1):
    """Pack len(d0) nodes (with per-half in-degree loads) into POOL_T
    tiles of 128 slots, balancing both halves' loads (greedy LPT on
    max(L0+d0, L1+d1) with a slot-count cap).  Returns tile per node."""
    n = len(d0)
    assert n == POOL_T * P
    order = np.argsort(-(d0 + d1), kind="stable")
    L0 = np.zeros(POOL_T)
    L1 = np.zeros(POOL_T)
    cnt = np.zeros(POOL_T, np.int64)
    tile_of = np.empty(n, np.int64)
    for i in order:
        c0 = L0 + d0[i]
        c1 = L1 + d1[i]
        score = np.maximum(c0, c1)
        score[cnt >= P] = 1e18
        t = int(np.argmin(score))
        tile_of[i] = t
        L0[t] = c0[t]
        L1[t] = c1[t]
        cnt[t] += 1
    return tile_of


def _preprocess(x, edge_index, edge_weight, W, b):
    """Shard + reorganize inputs on host. Returns (cfg, in_maps, g_idx)."""
    x = np.asarray(x, dtype=np.float32)
    W = np.asarray(W, dtype=np.float32)
    b = np.asarray(b, dtype=np.float32)
    ei = np.asarray(edge_index)
    ew = np.asarray(edge_weight, dtype=np.float32)

    N, C = x.shape
    F = W.shape[1]
    assert C % P == 0 and F == P
    CH = C // P
    PER = _ceil(N, NCORES)
    HALF = NCORES * POOL              # gather-table rows per pool table
    assert HALF <= 32768, "int16 gather index range exceeded"
    assert NP_ >= PER

    src = ei[0].astype(np.int64)
    dst = ei[1].astype(np.int64)

    o_src = src // PER
    lsrc = src - o_src * PER                    # 0..PER-1 within owner
    owner = dst // PER
    HALF_REAL = PER // 2
    half_e = (lsrc >= HALF_REAL).astype(np.int64)

    # per-node in-degree by src half -> packing loads
    d0 = np.zeros(N, np.int64)
    d1 = np.zeros(N, np.int64)
    np.add.at(d0, dst[half_e == 0], 1)
    np.add.at(d1, dst[half_e == 1], 1)

    # pack each (core, pool) into POOL_T tiles x 128 slots
    slot_of = np.zeros(N, np.int64)
    for c in range(NCORES):
        base = c * PER
        for pool in (0, 1):
            if pool == 0:
                nodes = np.arange(base, base + HALF_REAL)
            else:
                nodes = np.arange(base + HALF_REAL, base + PER)
            nfake = POOL - len(nodes)
            dd0 = np.concatenate([d0[nodes], np.zeros(nfake, np.int64)])
            dd1 = np.concatenate([d1[nodes], np.zeros(nfake, np.int64)])
            tile_of = _pack_pool(dd0, dd1)
            # slot within pool: tile*128 + fill position
            slot_local = np.empty(len(tile_of), np.int64)
            for t in range(POOL_T):
                m = np.nonzero(tile_of == t)[0]
                slot_local[m] = t * P + np.arange(len(m))
            slot_of[nodes] = pool * POOL + slot_local[:len(nodes)]

    slot_src = slot_of[src]
    gsrc = o_src * POOL + (slot_src % POOL)     # row within pool table
    half = slot_src // POOL                     # which table (0/1)
    slot_dst = slot_of[dst]
    tile_g = slot_dst // P
    dloc = slot_dst % P

    # per-core counts per (tile, half) -> unified block structure
    cnt = np.zeros((NCORES, NT, 2), np.int64)
    np.add.at(cnt, (owner, tile_g, half), 1)
    nb = _ceil(cnt, P).max(axis=0)              # [NT, 2]
    for t in range(NT):
        for h in (0, 1):
            if nb[t][h] == 0:
                nb[t][h] = 1

    # block stream: sweep all tiles' h0 runs (table A), then all h1 runs
    # (table B).  The h0 sweep streams while AllGather-B is still in
    # flight; each tile's h0 partial is parked in SBUF between sweeps.
    blocks = []                                  # (tile, half)
    base_blk = np.zeros((NT, 2), np.int64)
    for h in (0, 1):
        for t in range(NT):
            base_blk[t, h] = len(blocks)
            blocks.extend([(t, h)] * int(nb[t, h]))
    NBLK = len(blocks)
    NIDX = NBLK * P

    # calls: chunk maximal same-half runs into <=MAXBLK pieces
    calls = []                                   # (half, b0, nblk)
    i = 0
    while i < NBLK:
        h = blocks[i][1]
        j = i
        while j < NBLK and blocks[j][1] == h:
            j += 1
        k = i
        while k < j:
            n = min(MAXBLK, j - k)
            calls.append((h, k, n))
            k += n
        i = j
    seg_first = {}
    seg_last = {}
    for bi, (t, h) in enumerate(blocks):
        seg_first.setdefault((t, h), bi)
        seg_last[(t, h)] = bi

    # degree-table width (uniform across cores): max in-degree + 1 self
    deg_cnt = np.bincount(dst, minlength=N)
    PW = int(deg_cnt.max()) + 1

    import ml_dtypes
    in_maps = []
    B128 = np.tile(b[None, :], (P, 1)).astype(np.float32)
    IDENT = np.eye(P, dtype=np.float32).astype(ml_dtypes.bfloat16)

    base_flat = base_blk.reshape(-1)
    for c in range(NCORES):
        m = owner == c
        s_c = gsrc[m]
        h_c = half[m]
        t_c = tile_g[m]
        dl_c = dloc[m]
        w_c = ew[m]
        ld_c = slot_dst[m]

        # ---- edge stream positions (ascending gsrc within each bin for
        # HBM row locality during the gather) --------------------------
        key = t_c * 2 + h_c
        order = np.argsort(key * (1 << 15) + s_c, kind="stable")
        sk = key[order]
        grp_off = np.arange(len(sk)) - np.searchsorted(sk, sk)
        pos = base_flat[sk] * P + grp_off        # position in edge stream

        relidx = np.zeros(NIDX, np.int16)
        relidx[pos] = s_c[order].astype(np.int16)
        idx16 = np.ascontiguousarray(
            np.tile(relidx.reshape(NIDX // 16, 16).T, (P // 16, 1)))
        # host-built one-hot S blocks: S[e, dst_local] = w_e, laid out
        # [P (edge slot), NBLK*P] with block bi at cols [bi*P,(bi+1)*P)
        sblk = np.zeros((P, NBLK * P), np.float32)
        sblk[pos % P, (pos // P) * P + dl_c[order]] = w_c[order]
        sblk = sblk.astype(ml_dtypes.bfloat16)

        # ---- degree table [P, NT*PW]: row l%128, cols (l//128)*PW+j ---
        wpad = np.zeros((P, NT * PW), np.float32)
        o2 = np.argsort(ld_c, kind="stable")
        lds = ld_c[o2]
        ws = w_c[o2]
        off2 = np.arange(len(lds)) - np.searchsorted(lds, lds)
        wpad[lds % P, (lds // P) * PW + off2] = ws
        alln = np.arange(NP_)
        wpad[alln % P, (alln // P) * PW + np.minimum(
            np.bincount(ld_c, minlength=NP_), PW - 1)] = 1.0  # self loop

        # ---- xT shard [C, NP_] (node rows permuted to slots) ----------
        lo = c * PER
        hi = min((c + 1) * PER, N)
        xc = np.zeros((NP_, C), np.float32)
        xc[slot_of[lo:hi]] = x[lo:hi]
        xT = np.ascontiguousarray(xc.T).astype(ml_dtypes.bfloat16)

        in_maps.append({
            "xT": xT,
            "w_in": W.astype(ml_dtypes.bfloat16),
            "bias128": B128,
            "ident": IDENT,
            "wpad": wpad,
            "sblk": sblk,
            "idx16": idx16,
        })

    cfg = dict(N=N, C=C, F=F, CH=CH, PER=PER, HALF=HALF,
               NBLK=NBLK, NIDX=NIDX, PW=PW,
               nb=tuple(map(tuple, nb.tolist())),
               blocks=tuple(blocks), calls=tuple(calls),
               seg_first=tuple(sorted(seg_first.items())),
               seg_last=tuple(sorted(seg_last.items())))
    # full-output gather index: node -> row in concat-of-core outputs
    g_idx = (np.arange(N) // PER) * NP_ + slot_of
    return cfg, in_maps, g_idx


def _build(cfg):
    _import_concourse()
    from concourse import bacc, mybir, tile
    dt = mybir.dt
    Alu = mybir.AluOpType
    Act = mybir.ActivationFunctionType
    X = mybir.AxisListType.X

    C, F, CH = cfg["C"], cfg["F"], cfg["CH"]
    HALF, NBLK, NIDX, PW = cfg["HALF"], cfg["NBLK"], cfg["NIDX"], cfg["PW"]
    blocks = cfg["blocks"]
    calls = cfg["calls"]
    seg_first = dict(cfg["seg_first"])
    seg_last = dict(cfg["seg_last"])

    nc = bacc.Bacc("TRN2", target_bir_lowering=False, debug=False,
                   num_devices=NCORES, num_swdge_queues=4)

    xT_d = nc.dram_tensor("xT", [C, NP_], dt.bfloat16, kind="ExternalInput")
    W_d = nc.dram_tensor("w_in", [C, F], dt.bfloat16, kind="ExternalInput")
    B_d = nc.dram_tensor("bias128", [P, F], dt.float32, kind="ExternalInput")
    I_d = nc.dram_tensor("ident", [P, P], dt.bfloat16, kind="ExternalInput")
    wpad_d = nc.dram_tensor("wpad", [P, NT * PW], dt.float32,
                            kind="ExternalInput")
    sblk_d = nc.dram_tensor("sblk", [P, NBLK * P], dt.bfloat16,
                            kind="ExternalInput")
    idx_d = nc.dram_tensor("idx16", [P, NIDX // 16], dt.int16,
                           kind="ExternalInput")
    out_d = nc.dram_tensor("out", [NP_, F], dt.float32, kind="ExternalOutput")
    hs_sh = [
        nc.dram_tensor("hs_shA", [POOL, F], dt.bfloat16),
        nc.dram_tensor("hs_shB", [POOL, F], dt.bfloat16),
    ]
    hs_ag = [
        nc.dram_tensor("hs_agA", [HALF, F], dt.bfloat16, addr_space="Shared"),
        nc.dram_tensor("hs_agB", [HALF, F], dt.bfloat16, addr_space="Shared"),
    ]
    # gather from plain-DRAM copies — SWDGE reads from Shared space are slow
    hs_tab = [
        nc.dram_tensor("hs_fullA", [HALF, F], dt.bfloat16),
        nc.dram_tensor("hs_fullB", [HALF, F], dt.bfloat16),
    ]

    with tile.TileContext(nc) as tc:
        with (
            tc.tile_pool(name="const", bufs=1) as cpool,
            tc.tile_pool(name="psum", bufs=8, space="PSUM") as ppool,
            tc.tile_pool(name="work", bufs=8) as wpool,
            tc.tile_pool(name="gather", bufs=32) as gpool,
            tc.tile_pool(name="sbuild", bufs=12) as spool,
            tc.tile_pool(name="xt", bufs=4) as xpool,
        ):
            # ---------------- const loads ------------------------------
            W_sb = []
            for ch in range(CH):
                t2 = cpool.tile([P, F], dt.bfloat16, tag=f"W{ch}")
                nc.sync.dma_start(t2[:], W_d[ch * P:(ch + 1) * P, :])
                W_sb.append(t2)
            B_sb = cpool.tile([P, F], dt.float32, tag="B")
            nc.sync.dma_start(B_sb[:], B_d[:])
            I_sb = cpool.tile([P, P], dt.bfloat16, tag="I")
            nc.sync.dma_start(I_sb[:], I_d[:])
            wpad_sb = cpool.tile([P, NT * PW], dt.float32, tag="wpad")
            nc.sync.dma_start(wpad_sb[:], wpad_d[:])
            idx_sb = cpool.tile([P, NIDX // 16], dt.int16, tag="idx")
            nc.sync.dma_start(idx_sb[:], idx_d[:])
            part_sb = cpool.tile([P, NT * F], dt.float32, tag="part")
            hs_tiles = [cpool.tile([P, F], dt.bfloat16, tag=f"hs{t}",
                                   name=f"hs{t}")
                        for t in range(NT)]
            dinv_sb = cpool.tile([P, NT], dt.float32, tag="dinv")

            # ---------------- phase 1: degrees -------------------------
            degt = wpool.tile([P, NT], dt.float32, tag="deg")
            for t in range(NT):
                nc.vector.reduce_sum(degt[:, t:t + 1],
                                     wpad_sb[:, t * PW:(t + 1) * PW], X)
            rec = wpool.tile([P, NT], dt.float32, tag="rec")
            nc.vector.reciprocal(rec[:], degt[:])
            nc.scalar.activation(dinv_sb[:], rec[:], Act.Sqrt)

            # ---------------- phase 2: h = xW, hs, hsb -----------------
            # xT loaded in octets of 8 tiles (fewer, bigger DMAs);
            # per-pool: finish pool's tiles then AllGather that pool.
            def phase2_tiles(t0, t1):
                t = t0
                while t < t1:
                    span = min(8, t1 - t)
                    xts = []
                    for ch in range(CH):
                        xt_t = xpool.tile([P, 8 * P], dt.bfloat16, tag="xt")
                        nc.sync.dma_start(
                            xt_t[:, :span * P],
                            xT_d[ch * P:(ch + 1) * P,
                                 t * P:(t + span) * P])
                        xts.append(xt_t)
                    for j in range(span):
                        tt = t + j
                        ph = ppool.tile([P, F], dt.float32, tag="psum")
                        for ch in range(CH):
                            nc.tensor.matmul(
                                ph[:], xts[ch][:, j * P:(j + 1) * P],
                                W_sb[ch][:], start=(ch == 0),
                                stop=(ch == CH - 1))
                        hsbf = hs_tiles[tt]
                        nc.vector.tensor_scalar(
                            hsbf[:], ph[:], dinv_sb[:, tt:tt + 1],
                            None, Alu.mult)
                        pool, tl = divmod(tt, POOL_T)
                        nc.scalar.dma_start(
                            hs_sh[pool][tl * P:(tl + 1) * P, :], hsbf[:])
                    t += span

            for hh in (0, 1):
                phase2_tiles(hh * POOL_T, (hh + 1) * POOL_T)
                # -------- phase 3: AllGather this pool (<1MB/rank) -----
                nc.gpsimd.collective_compute(
                    "AllGather", Alu.bypass,
                    replica_groups=[list(range(NCORES))],
                    ins=[hs_sh[hh].ap().opt()],
                    outs=[hs_ag[hh].ap().opt()],
                )
                # copies off the gpsimd queue so they never head-of-line
                # block the gather stream.  copy-A on sync (ahead of the
                # S-slab stream), copy-B on scalar (ahead of out-writes).
                eng = nc.sync if hh == 0 else nc.scalar
                eng.dma_start(hs_tab[hh].ap(), hs_ag[hh].ap())

            # ---------------- phase 4: gather + segment matmul ---------
            agg = {}
            for ci, (h, b0, nbc) in enumerate(calls):
                gb = gpool.tile([P, MAXBLK, F], dt.bfloat16, tag="gb")
                nc.gpsimd.dma_gather(
                    gb[:, :nbc, :],
                    hs_tab[h].ap(),
                    idx_sb[:, b0 * (P // 16):(b0 + nbc) * (P // 16)],
                    nbc * P, nbc * P, F, single_packet=False,
                    queue_num=ci % 4)
                sb = spool.tile([P, MAXBLK, P], dt.bfloat16, tag="S")
                nc.sync.dma_start(
                    sb[:, :nbc, :],
                    sblk_d[:, b0 * P:(b0 + nbc) * P])
                for j in range(nbc):
                    bi = b0 + j
                    t, hh = blocks[bi]
                    S = sb[:, j, :]
                    first = bi == seg_first[(t, hh)]
                    last = bi == seg_last[(t, hh)]
                    if first:
                        agg[(t, hh)] = ppool.tile([P, F], dt.float32,
                                                  tag="psum",
                                                  name=f"agg{t}_{hh}")
                    a = agg[(t, hh)]
                    inj = hh == 0 and first
                    nc.tensor.matmul(a[:], S, gb[:, j, :], start=first,
                                     stop=(last and not inj))
                    if inj:
                        # self-loop: agg += I^T @ hs_t  (adds dinv*h_t)
                        nc.tensor.matmul(a[:], I_sb[:], hs_tiles[t][:],
                                         start=False, stop=last)
                    if not last:
                        continue
                    if hh == 0:
                        # park sweep-A partial: dinv*(aggA+hs) + b
                        nc.vector.scalar_tensor_tensor(
                            part_sb[:, t * F:(t + 1) * F], a[:],
                            dinv_sb[:, t:t + 1], B_sb[:],
                            Alu.mult, Alu.add)
                    else:
                        res = wpool.tile([P, F], dt.float32, tag="res")
                        nc.vector.scalar_tensor_tensor(
                            res[:], a[:], dinv_sb[:, t:t + 1],
                            part_sb[:, t * F:(t + 1) * F],
                            Alu.mult, Alu.add)
                        ot = wpool.tile([P, F], dt.float32, tag="ot")
                        nc.scalar.activation(ot[:], res[:], Act.Relu)
                        nc.scalar.dma_start(out_d[t * P:(t + 1) * P, :],
                                            ot[:])

    nc.compile()
    return nc


# knobs test.py can flip
TRACE = False
LAST_EXEC_NS = None
LAST_TRACE_PATH = None


def _cfg_key(cfg):
    return (cfg["N"], cfg["C"], cfg["F"], cfg["NBLK"], cfg["PW"],
            cfg["nb"], cfg["calls"])


def kernel(x, edge_index, edge_weight, W, b):
    global LAST_EXEC_NS, LAST_TRACE_PATH
    _import_concourse()
    from concourse import bass_utils

    cfg, in_maps, g_idx = _preprocess(x, edge_index, edge_weight, W, b)
    key = _cfg_key(cfg)
    nc = _CACHE.get(key)
    if nc is None:
        nc = _build(cfg)
        _CACHE[key] = nc

    res = bass_utils.run_bass_kernel_spmd(
        nc, in_maps, core_ids=list(range(NCORES)), trace=TRACE)
    LAST_EXEC_NS = res.exec_time_ns
    if res.instructions_and_trace is not None:
        LAST_TRACE_PATH = res.instructions_and_trace[1]

    flat = np.concatenate([res.results[c]["out"] for c in range(NCORES)],
                          axis=0)
    return np.ascontiguousarray(flat[g_idx])
